# revision 1
# baseline (speedup 1.0000x reference)
"""Swin-style windowed attention (b=16, n=1024, 8 heads x 32, relative
position bias) for 8 Trainium2 NeuronCores, data-parallel over batch.

Host prep: transpose x, fold softmax scale into w_q, expand exp(bias^T) to a
bf16 table, pack w_out (+ b_out row) for the head-packed output projection.

Per-core program (2 batches):
  phase 1: qT/kT (feature-major, f32) and v (token-major, bf16 + ones col)
           via f32r matmuls off xT.
  phase 2: per (head-group, batch, q-block): for each k-tile, row-packed
           QK^T into PSUM -> ACT exp -> DVE mult by exp(bias^T) -> PV
           matmuls (M=33: 32 v-cols + ones col => denominator row).
           Then reciprocal + K=1 broadcast matmul + normalize.
  phase 3: y = outT_packed.T @ w_packed (b_out folded in), DMA out.
"""

import dataclasses

import numpy as np


def _ensure_path():
    try:
        import concourse.bass  # noqa: F401
    except ImportError:
        import sys

        for p in ("/opt/trn_rl_repo", "/root/.axon_site/_ro/trn_rl_repo"):
            if p not in sys.path:
                sys.path.insert(0, p)


_ensure_path()

import concourse.bass as bass  # noqa: E402
import concourse.tile as tile  # noqa: E402
from concourse import mybir  # noqa: E402
from concourse.bass_utils import run_bass_kernel_spmd  # noqa: E402

HEADS = 8
DH = 32
INP = 512
OUP = 512
N = 1024
B = 16
NCORES = 8
BPC = B // NCORES  # batches per core
T = BPC * N  # tokens per core
TABLE = 3969

F32 = mybir.dt.float32
FR = mybir.dt.float32r
BF = mybir.dt.bfloat16
Exp = mybir.ActivationFunctionType.Exp
Ln = mybir.ActivationFunctionType.Ln


# walrus on this build rejects more than ONE sync-wait per instruction
# (any engine/struct) -- probed: cap=2 fails even on Matmult.
_CTRL_OPS = ()
_COMPUTE_CAP = 1


def _split_waits(nc, cap=1):
    """Split instructions with too many semaphore waits into same-engine
    NoOp chains."""
    n = 0
    for _, bb_wrap in nc.bb_map.items():
        bb = bb_wrap.bb if hasattr(bb_wrap, "bb") else bb_wrap
        new_list = []
        changed = False
        for inst in bb.instructions:
            si = inst.sync_info
            cap = _COMPUTE_CAP
            if si is not None and si.on_wait and len(si.on_wait) > cap:
                waits = list(si.on_wait)
                rest, head = waits[:-cap], waits[-cap:]
                for i in range(0, len(rest), cap):
                    nop = mybir.InstNoOp(name=f"{inst.name}_wsplit{i}")
                    nop.engine = inst.engine
                    nop.sync_info = mybir.SyncInfo(
                        on_wait=rest[i : i + cap], on_update=[]
                    )
                    nc.register_instruction(nop, overwrite=True)
                    new_list.append(nop)
                    n += 1
                inst.sync_info = mybir.SyncInfo(
                    on_wait=head, on_update=list(si.on_update)
                )
                changed = True
            new_list.append(inst)
        if changed:
            bb.instructions = new_list
    return n


def _emit_body(nc, tc, es, aps):
    """One repetition of the full per-core computation."""
    from contextlib import ExitStack

    xT_d, wqk_d, wv_d, eb_d, wpk_d, y_d = aps

    KT = N // 128  # 8 k-tiles per batch

    # ---- persistent-for-this-rep pools (released last, LIFO) ---------------
    p_on = es.enter_context(tc.tile_pool(name="on", bufs=8))  # outT normalized
    # eb pool created before phase-1 pools so its SBUF range doesn't overlap
    # xT and its DMAs can stream during phase 1.
    p_eb = es.enter_context(tc.tile_pool(name="eb", bufs=2))
    p_qk = es.enter_context(tc.tile_pool(name="qk", bufs=4))
    p_v = es.enter_context(tc.tile_pool(name="v", bufs=1))
    on_tiles = {}

    with ExitStack() as ph1:
        p_x = ph1.enter_context(tc.tile_pool(name="xT", bufs=4))
        p_w = ph1.enter_context(tc.tile_pool(name="w", bufs=4))
        p_ps1 = ph1.enter_context(tc.tile_pool(name="ps1", bufs=4, space="PSUM"))

        xT = []
        for dm in range(4):
            t = p_x.tile([128, T], FR, tag="xT", name=f"xT{dm}")
            nc.sync.dma_start(t[:], xT_d[dm * 128 : (dm + 1) * 128, :])
            xT.append(t)
        wqk = []
        wv = []
        for dm in range(4):
            t = p_w.tile([128, 512], FR, tag="wqk", name=f"wqk{dm}")
            nc.sync.dma_start(t[:], wqk_d[dm * 128 : (dm + 1) * 128, :])
            wqk.append(t)
            t = p_w.tile([128, 256], FR, tag="wv", name=f"wv{dm}")
            nc.sync.dma_start(t[:], wv_d[dm * 128 : (dm + 1) * 128, :])
            wv.append(t)

        # qT/kT feature-major: tile ft: 0=q heads0-3, 1=q heads4-7, 2=k h0-3, 3=k h4-7
        qk = [p_qk.tile([128, T], FR, tag="qk", name=f"qk{i}") for i in range(4)]
        # v token-major: per token-tile, 4 head-pairs of 97 cols:
        # [v_h(32) | ones | zeros(31) | v_h+1(32) | ones], so a single M=97
        # PV matmul per pair yields the {0-32, 64-96} psum row layout with
        # denominator rows at 32/96 (one accumulation group per bank).
        v_aug = p_v.tile([128, (T // 128) * 388], BF)
        nc.vector.memset(v_aug[:], 0.0)
        # ones columns at pair*97 + {32, 96}
        va4 = v_aug[:].rearrange("p (t pr c) -> p t pr c", t=T // 128, pr=4)
        nc.vector.memset(va4[:, :, :, 32:33], 1.0)
        nc.vector.memset(va4[:, :, :, 96:97], 1.0)

        for ft in range(4):
            for tb in range(T // 512):
                ps = p_ps1.tile([128, 512], F32, tag="ps1")
                for dm in range(4):
                    nc.tensor.matmul(
                        ps[:],
                        wqk[dm][:, ft * 128 : (ft + 1) * 128],
                        xT[dm][:, tb * 512 : (tb + 1) * 512],
                        start=(dm == 0),
                        stop=(dm == 3),
                    )
                nc.vector.tensor_copy(qk[ft][:, tb * 512 : (tb + 1) * 512], ps[:])

        for tt in range(T // 128):
            ps = p_ps1.tile([128, 256], F32, tag="ps1")
            for dm in range(4):
                nc.tensor.matmul(
                    ps[:],
                    xT[dm][:, tt * 128 : (tt + 1) * 128],
                    wv[dm][:],
                    start=(dm == 0),
                    stop=(dm == 3),
                )
            # dst col = pr*97 + e*64 + d  (head h = 2*pr + e)
            dstp = v_aug[:, tt * 388 : (tt + 1) * 388].rearrange(
                "p (pr c) -> p pr c", pr=4
            )
            srcp = ps[:].rearrange("p (pr e d) -> p pr e d", pr=4, e=2)
            for e in range(2):
                nc.scalar.copy(dstp[:, :, e * 64 : e * 64 + 32], srcp[:, :, e, :])

    # ---- phase 2: attention ------------------------------------------------
    with ExitStack() as ph2:
        p_p = ph2.enter_context(tc.tile_pool(name="pexp", bufs=3))
        p_ph = ph2.enter_context(tc.tile_pool(name="phat", bufs=3))
        p_dots = ph2.enter_context(tc.tile_pool(name="dots", bufs=3, space="PSUM"))
        p_po = ph2.enter_context(tc.tile_pool(name="po", bufs=1, space="PSUM"))
        p_r = ph2.enter_context(tc.tile_pool(name="recip", bufs=2))

        for hg in range(2):
            # two 4 MiB mega-tiles per head-group (32 KiB DMA rows):
            # half h2 covers kt in [4*h2, 4*h2+4); free index inside:
            # ((kt%4)*4 + j)*1024 + qt  for j = head-in-group.
            eb = []
            for h2 in range(2):
                t = p_eb.tile([128, 16384], BF, tag="eb", name=f"eb{hg}_{h2}")
                nc.sync.dma_start(t[:], eb_d[hg, h2])
                eb.append(t)
            for b in range(BPC):
                for qb in range(2):
                    q0 = b * N + qb * 512
                    po = p_po.tile([128, 1024], F32, tag="po")
                    for kt in range(KT):
                        for hp in range(2):  # head-pair within group
                            dots = p_dots.tile([128, 1024], F32, tag="dots")
                            for jj in range(2):
                                j = hp * 2 + jj  # head within group
                                h = 4 * hg + j
                                pb = (h % 4) * 32
                                nc.tensor.matmul(
                                    dots[:, jj * 512 : (jj + 1) * 512],
                                    qk[2 + hg][pb : pb + 32, b * N + kt * 128 : b * N + kt * 128 + 128],
                                    qk[hg][pb : pb + 32, q0 : q0 + 512],
                                    start=True,
                                    stop=True,
                                    tile_position=(pb, 0),
                                )
                            P = p_p.tile([128, 1024], BF, tag="pexp")
                            nc.scalar.activation(P[:], dots[:], Exp)
                            Ph = p_ph.tile([128, 1024], BF, tag="phat")
                            ebsl = eb[kt // 4][:].rearrange(
                                "p (h q) -> p h q", h=16
                            )[
                                :,
                                (kt % 4) * 4 + hp * 2 : (kt % 4) * 4 + hp * 2 + 2,
                                qb * 512 : qb * 512 + 512,
                            ]
                            psl = P[:].rearrange("p (h q) -> p h q", h=2)
                            phsl = Ph[:].rearrange("p (h q) -> p h q", h=2)
                            nc.vector.tensor_mul(phsl, psl, ebsl)
                            # PV: per head, M=33 matmul ([v_h | ones] ->
                            # den in row 32/96). Two heads share a bank on
                            # disjoint partition ranges {0-32},{64-96}; HW
                            # start=True clears has_written only for the
                            # instruction's own footprint (probed), so each
                            # range is its own accumulation stream.
                            pr = 2 * hg + hp
                            base = (b * KT + kt) * 388 + pr * 97
                            for jj in range(2):
                                cb = jj * 64
                                nc.tensor.matmul(
                                    po[cb : cb + 33, hp * 512 : (hp + 1) * 512],
                                    v_aug[:, base + jj * 64 : base + jj * 64 + 33],
                                    Ph[:, jj * 512 : (jj + 1) * 512],
                                    start=(kt == 0),
                                    stop=(kt == KT - 1),
                                    tile_position=(0, cb),
                                    skip_group_check=True,
                                )
                    # normalize: reciprocal of den rows (32, 96), broadcast
                    # via K=1 matmuls, multiply.
                    # 1/den via exp(-ln(den)) on ACT (DVE reciprocal is
                    # ~8 cyc/elem; ACT ops are FD-driven): ln reads the den
                    # rows straight from PSUM, the ln-rows get broadcast
                    # across partitions by a free-dim step-0 DMA (idle
                    # gpsimd/SWDGE ring), then one exp(-x) per range.
                    r = p_r.tile([128, 1024], F32, tag="recip")
                    nc.scalar.activation(r[32:33, :], po[32:33, :], Ln)
                    nc.scalar.activation(r[96:97, :], po[96:97, :], Ln)
                    rb = p_r.tile([128, 1024], F32, tag="rbcast")
                    for half in range(2):
                        sb = 32 + half * 64
                        src = dataclasses.replace(
                            r[sb : sb + 1, :], ap=[[1024, 1], [0, 33], [1, 1024]]
                        )
                        nc.gpsimd.dma_start(rb[half * 64 : half * 64 + 33, :], src)
                    nc.scalar.activation(rb[0:33, :], rb[0:33, :], Exp, scale=-1.0)
                    nc.scalar.activation(rb[64:97, :], rb[64:97, :], Exp, scale=-1.0)

                    on = p_on.tile([128, 1024], BF, tag="on")
                    nc.vector.memset(on[:], 0.0)
                    # rows 32/96 become den*recip ~= 1.0 after the normalize;
                    # wpk row 32 of group 0 carries b_out.
                    nc.vector.tensor_mul(on[0:33, :], po[0:33, :], rb[0:33, :])
                    nc.vector.tensor_mul(on[64:97, :], po[64:97, :], rb[64:97, :])
                    on_tiles[(hg, b, qb)] = on

    # ---- phase 3: output projection ---------------------------------------
    with ExitStack() as ph3:
        p_wp = ph3.enter_context(tc.tile_pool(name="wpk", bufs=4))
        p_psy = ph3.enter_context(tc.tile_pool(name="psy", bufs=2, space="PSUM"))
        p_y = ph3.enter_context(tc.tile_pool(name="ysb", bufs=2))

        wpk = []
        for g in range(4):
            t = p_wp.tile([128, 512], BF, tag="wpk", name=f"wpk{g}")
            nc.sync.dma_start(t[:], wpk_d[g])
            wpk.append(t)

        for b in range(BPC):
            for qb in range(2):
                ysb = p_y.tile([128, 4 * OUP], F32, tag="ysb")
                for t4 in range(4):
                    psy = p_psy.tile([128, 512], F32, tag="psy")
                    for g in range(4):
                        hg, hp = g // 2, g % 2
                        on = on_tiles[(hg, b, qb)]
                        nc.tensor.matmul(
                            psy[:],
                            on[:, hp * 512 + t4 * 128 : hp * 512 + t4 * 128 + 128],
                            wpk[g][:],
                            start=(g == 0),
                            stop=(g == 3),
                        )
                    nc.vector.tensor_copy(
                        ysb[:, t4 * 512 : (t4 + 1) * 512], psy[:]
                    )
                nc.scalar.dma_start(y_d[b, qb], ysb[:])


def build_program(reps=1):
    nc = bass.Bass("TRN2", target_bir_lowering=False, debug=False, num_devices=NCORES)
    xT_d = nc.dram_tensor("xT", [INP, T], FR, kind="ExternalInput").ap()
    wqk_d = nc.dram_tensor("wqk", [INP, 512], FR, kind="ExternalInput").ap()
    wv_d = nc.dram_tensor("wv", [INP, 256], FR, kind="ExternalInput").ap()
    eb_d = nc.dram_tensor("eb", [2, 2, 128, 16384], BF, kind="ExternalInput").ap()
    wpk_d = nc.dram_tensor("wpk", [4, 128, 512], BF, kind="ExternalInput").ap()
    y_d = nc.dram_tensor("y", [BPC, 2, 128, 4 * OUP], F32, kind="ExternalOutput").ap()
    aps = (xT_d, wqk_d, wv_d, eb_d, wpk_d, y_d)

    from contextlib import ExitStack

    with tile.TileContext(nc) as tc:
        for _ in range(reps):
            with ExitStack() as es:
                _emit_body(nc, tc, es, aps)

    _split_waits(nc, cap=1)
    return nc


def _relative_index():
    ii, jj = np.meshgrid(np.arange(32), np.arange(32), indexing="ij")
    coords = np.stack([ii.reshape(-1), jj.reshape(-1)])
    rel = coords[:, :, None] - coords[:, None, :]
    return ((rel[0] + 31) * 63 + (rel[1] + 31)).reshape(-1)


def prepare_inputs(x, w_qkv, bias_table, w_out, b_out):
    """Host-side prep: returns per-core in_maps."""
    import ml_dtypes

    bf16 = ml_dtypes.bfloat16
    scale = DH ** -0.5

    wqk = np.ascontiguousarray(w_qkv[:, :512]).astype(np.float32).copy()
    wqk[:, :256] *= scale
    wv = np.ascontiguousarray(w_qkv[:, 512:]).astype(np.float32)

    idx = np.clip(_relative_index(), 0, TABLE - 1)
    bias = bias_table[idx].reshape(N, N, HEADS).astype(np.float32)  # [q, k, h]
    ebT = np.exp(bias).transpose(1, 0, 2)  # [k, q, h]
    # mega-tile layout [hg, half, p, (ktl, j, qt)]: k = (half*4 + ktl)*128 + p,
    # h = hg*4 + j
    eb = (
        ebT.reshape(2, 4, 128, N, 2, 4)  # [half, ktl, p, qt, hg, j]
        .transpose(4, 0, 2, 1, 5, 3)  # [hg, half, p, ktl, j, qt]
        .reshape(2, 2, 128, 16384)
        .astype(bf16)
    )

    wpk = np.zeros((4, 128, 512), np.float32)
    for g in range(4):
        wpk[g, 0:32] = w_out[(2 * g) * 32 : (2 * g) * 32 + 32]
        wpk[g, 64:96] = w_out[(2 * g + 1) * 32 : (2 * g + 1) * 32 + 32]
    wpk[0, 32] = b_out
    wpk = wpk.astype(bf16)

    in_maps = []
    for c in range(NCORES):
        xc = x[c * BPC : (c + 1) * BPC].reshape(T, INP)
        xT = np.ascontiguousarray(xc.T.astype(np.float32))
        in_maps.append({"xT": xT, "wqk": wqk, "wv": wv, "eb": eb, "wpk": wpk})
    return in_maps


_NC_CACHE = {}


def kernel(x, w_qkv, bias_table, w_out, b_out):
    in_maps = prepare_inputs(x, w_qkv, bias_table, w_out, b_out)
    if 1 not in _NC_CACHE:
        _NC_CACHE[1] = build_program(reps=1)
    nc = _NC_CACHE[1]
    res = run_bass_kernel_spmd(nc, in_maps, list(range(NCORES)), trace=False)
    # y DRAM layout [b, qb, p, t4*512+o]: token = b*1024 + qb*512 + t4*128 + p
    y = np.concatenate(
        [
            res.results[c]["y"]
            .reshape(BPC, 2, 128, 4, OUP)
            .transpose(0, 1, 3, 2, 4)
            .reshape(BPC, N, OUP)
            for c in range(NCORES)
        ],
        axis=0,
    )
    return y.astype(np.float32)

